# revision 2
# baseline (speedup 1.0000x reference)
"""Batched MoE (top-2, 8 experts) on 8 Trainium2 NeuronCores.

Strategy: expert-parallel — core e owns expert e's weights (w1/w2/w3) and
processes the tokens routed to it. Routing (sort by expert / capacity
padding) and the combine (weighting by gate prob + scatter-add over top-k)
are cheap O(tokens) index ops done on host; all matmul FLOPs run on device.

Device dataflow per core (capacity C columns, zero-padded):
    xt  = X_e^T               [1024, C]   (d on partitions)
    GT  = w1^T @ xt           [4096, C]   lhsT = w1 tiles (natural layout)
    VT  = w2^T @ xt           [4096, C]
    HT  = silu(GT) * VT       [4096, C]
    OT  = w3^T @ HT           [1024, C]   lhsT = w3 tiles (natural layout)
All operands enter the PE in their natural DRAM layout — no transposes.
Matmuls run in float32r (TF32-class, 1 cycle/row at free-dim >= 256, 4x
faster than plain fp32). The f dimension is processed in chunks of 512 so
weights stream through SBUF; OT accumulates across chunks in SBUF via DVE.
"""

import numpy as np

N_EXPERTS = 8
D_MODEL = 1024
D_FF = 4096
FC = 512                # d_ff chunk streamed through SBUF
KT = D_MODEL // 128     # 8 k-tiles (contraction d)
JT = FC // 128          # 4 f-tiles per chunk
NCH = D_FF // FC        # 8 chunks
MT = D_MODEL // 128     # 8 output d-tiles

_program_cache = {}


def _col_chunks(C):
    """Split C columns into <=512 pieces, all >=256 when possible (float32r
    runs at 1 cycle/row only for free dim >= 256)."""
    if C <= 512:
        return [(0, C)]
    n = (C + 511) // 512
    base = C // n
    rem = C - base * n
    out = []
    off = 0
    for i in range(n):
        sz = base + (1 if i < rem else 0)
        out.append((off, sz))
        off += sz
    return out


def _build_program(C):
    import concourse.bacc as bacc
    import concourse.mybir as mybir
    from concourse.tile import TileContext

    F32R = mybir.dt.float32r
    F32 = mybir.dt.float32
    SILU = mybir.ActivationFunctionType.Silu
    ccs = _col_chunks(C)

    nc = bacc.Bacc()
    xt_d = nc.declare_dram_parameter("xt", [D_MODEL, C], F32R, isOutput=False)
    w1_d = nc.declare_dram_parameter("w1", [D_MODEL, D_FF], F32R, isOutput=False)
    w2_d = nc.declare_dram_parameter("w2", [D_MODEL, D_FF], F32R, isOutput=False)
    w3_d = nc.declare_dram_parameter("w3", [D_FF, D_MODEL], F32R, isOutput=False)
    ot_d = nc.declare_dram_parameter("ot", [D_MODEL, C], F32, isOutput=True)

    xt_r = xt_d.rearrange("(k p) c -> k p c", p=128)
    w1_r = w1_d.rearrange("(k p) f -> k p f", p=128)
    w2_r = w2_d.rearrange("(k p) f -> k p f", p=128)
    w3_r = w3_d.rearrange("(j p) d -> j p d", p=128)
    ot_r = ot_d.rearrange("(m p) c -> m p c", p=128)

    with TileContext(nc) as tc:
        with (
            tc.tile_pool(name="xtp", bufs=1) as xt_pool,
            tc.tile_pool(name="w12", bufs=2) as w12_pool,
            tc.tile_pool(name="w3p", bufs=2) as w3_pool,
            tc.tile_pool(name="htp", bufs=2) as ht_pool,
            tc.tile_pool(name="otp", bufs=1) as ot_pool,
            tc.tile_pool(name="tmp", bufs=4) as tmp_pool,
            tc.tile_pool(name="pg", bufs=2, space="PSUM") as pg_pool,
            tc.tile_pool(name="pv", bufs=2, space="PSUM") as pv_pool,
            tc.tile_pool(name="po", bufs=2, space="PSUM") as po_pool,
        ):
            xt_sb = []
            for k in range(KT):
                t = xt_pool.tile([128, C], F32R, tag=f"xt{k}", name=f"xt{k}")
                nc.sync.dma_start(out=t[:], in_=xt_r[k])
                xt_sb.append(t)
            ot_sb = [
                ot_pool.tile([128, C], F32, tag=f"ot{m}", name=f"ot{m}")
                for m in range(MT)
            ]

            for ch in range(NCH):
                f0 = ch * FC
                w1c, w2c = [], []
                for k in range(KT):
                    t1 = w12_pool.tile([128, FC], F32R, tag=f"w1k{k}", name=f"w1c{k}")
                    nc.sync.dma_start(out=t1[:], in_=w1_r[k][:, f0 : f0 + FC])
                    w1c.append(t1)
                    t2 = w12_pool.tile([128, FC], F32R, tag=f"w2k{k}", name=f"w2c{k}")
                    nc.sync.dma_start(out=t2[:], in_=w2_r[k][:, f0 : f0 + FC])
                    w2c.append(t2)
                w3c = []
                for jj in range(JT):
                    t3 = w3_pool.tile([128, D_MODEL], F32R, tag=f"w3j{jj}", name=f"w3c{jj}")
                    nc.sync.dma_start(out=t3[:], in_=w3_r[ch * JT + jj])
                    w3c.append(t3)

                hts = []
                for jj in range(JT):
                    ht_t = ht_pool.tile([128, C], F32R, tag=f"ht{jj}", name=f"ht{jj}")
                    js = slice(jj * 128, (jj + 1) * 128)
                    for c0, cl in ccs:
                        cs = slice(c0, c0 + cl)
                        pg = pg_pool.tile([128, cl], F32, tag="pg", name="pg")
                        pv = pv_pool.tile([128, cl], F32, tag="pv", name="pv")
                        for k in range(KT):
                            nc.tensor.matmul(
                                out=pg[:],
                                lhsT=w1c[k][:, js],
                                rhs=xt_sb[k][:, cs],
                                start=(k == 0),
                                stop=(k == KT - 1),
                            )
                        for k in range(KT):
                            nc.tensor.matmul(
                                out=pv[:],
                                lhsT=w2c[k][:, js],
                                rhs=xt_sb[k][:, cs],
                                start=(k == 0),
                                stop=(k == KT - 1),
                            )
                        st = tmp_pool.tile([128, cl], F32, tag="silu", name="st")
                        nc.scalar.activation(st[:], pg[:], SILU)
                        nc.vector.tensor_mul(out=ht_t[:, cs], in0=st[:], in1=pv[:])
                    hts.append(ht_t)

                for m in range(MT):
                    ms = slice(m * 128, (m + 1) * 128)
                    for c0, cl in ccs:
                        cs = slice(c0, c0 + cl)
                        po = po_pool.tile([128, cl], F32, tag="po", name="po")
                        for jj in range(JT):
                            nc.tensor.matmul(
                                out=po[:],
                                lhsT=w3c[jj][:, ms],
                                rhs=hts[jj][:, cs],
                                start=(jj == 0),
                                stop=(jj == JT - 1),
                            )
                        if ch == 0:
                            nc.vector.tensor_copy(out=ot_sb[m][:, cs], in_=po[:])
                        else:
                            nc.vector.tensor_add(
                                out=ot_sb[m][:, cs], in0=ot_sb[m][:, cs], in1=po[:]
                            )

            for m in range(MT):
                nc.sync.dma_start(out=ot_r[m], in_=ot_sb[m][:])

    nc.compile()
    return nc


def _get_program(C):
    if C not in _program_cache:
        _program_cache[C] = _build_program(C)
    return _program_cache[C]


def _run(nc, in_maps, trace=False):
    from concourse.bass_utils import run_bass_kernel_spmd

    last = None
    for attempt in range(3):
        try:
            return run_bass_kernel_spmd(
                nc, in_maps, list(range(N_EXPERTS)), trace=trace
            )
        except Exception as e:  # stale device state from a prior crash
            last = e
    raise last


def kernel(x, expert_indices, expert_weights, w1, w2, w3, _trace=False):
    x = np.ascontiguousarray(np.asarray(x, dtype=np.float32))
    expert_indices = np.asarray(expert_indices)
    expert_weights = np.asarray(expert_weights, dtype=np.float32)
    w1 = np.asarray(w1, dtype=np.float32)
    w2 = np.asarray(w2, dtype=np.float32)
    w3 = np.asarray(w3, dtype=np.float32)

    n_tokens, d_model = x.shape
    top_k = expert_indices.shape[1]
    n_experts = w1.shape[0]
    A = n_tokens * top_k

    flat_e = expert_indices.reshape(-1).astype(np.int64)
    flat_w = expert_weights.reshape(-1)
    tok_idx = np.repeat(np.arange(n_tokens), top_k)
    order = np.argsort(flat_e, kind="stable")
    s_tok = tok_idx[order]
    s_w = flat_w[order]
    counts = np.bincount(flat_e, minlength=n_experts)
    starts = np.concatenate([[0], np.cumsum(counts)[:-1]])

    C = int(counts.max())
    C = max(256, -(-C // 64) * 64)  # round up to multiple of 64

    xt = np.zeros((n_experts, d_model, C), np.float32)
    for e in range(n_experts):
        seg = s_tok[starts[e] : starts[e] + counts[e]]
        xt[e, :, : counts[e]] = x[seg].T

    nc = _get_program(C)
    in_maps = [
        {"xt": xt[e], "w1": w1[e], "w2": w2[e], "w3": w3[e]}
        for e in range(n_experts)
    ]
    res = _run(nc, in_maps, trace=_trace)

    y = np.empty((A, d_model), np.float32)
    for e in range(n_experts):
        ot = res.results[e]["ot"]
        y[starts[e] : starts[e] + counts[e]] = ot[:, : counts[e]].T
    y *= s_w[:, None]
    y_orig = np.empty_like(y)
    y_orig[order] = y
    out = y_orig.reshape(n_tokens, top_k, d_model).sum(axis=1, dtype=np.float32)
    if _trace:
        return out.astype(np.float32, copy=False), res
    return out.astype(np.float32, copy=False)


# revision 3
# speedup vs baseline: 1.0134x; 1.0134x over previous
"""Batched MoE (top-2, 8 experts) on 8 Trainium2 NeuronCores.

Strategy: expert-parallel — core e owns expert e's weights (w1/w2/w3) and
processes the tokens routed to it. Routing (sort by expert / capacity
padding) and the combine (weighting by gate prob + scatter-add over top-k)
are cheap O(tokens) index ops done on host; all matmul FLOPs run on device.

Device dataflow per core (capacity C columns, zero-padded):
    xt  = X_e^T               [1024, C]   (d on partitions)
    GT  = w1^T @ xt           [4096, C]   lhsT = w1 tiles (natural layout)
    VT  = w2^T @ xt           [4096, C]
    HT  = silu(GT) * VT       [4096, C]
    OT  = w3^T @ HT           [1024, C]   lhsT = w3 tiles (natural layout)
All operands enter the PE in their natural DRAM layout — no transposes.
Matmuls run in float32r (TF32-class, 1 cycle/row at free-dim >= 256, 4x
faster than plain fp32). The f dimension is processed in chunks of 512 so
weights stream through SBUF; OT accumulates across chunks in SBUF via DVE.
Phase B (OT accumulation) of chunk ch is issued after phase A of chunk
ch+1 so the PE never waits on the ACT/DVE epilogue that produces HT.
"""

import numpy as np

N_EXPERTS = 8
D_MODEL = 1024
D_FF = 4096
FC = 512                # d_ff chunk streamed through SBUF
KT = D_MODEL // 128     # 8 k-tiles (contraction d)
JT = FC // 128          # 4 f-tiles per chunk
NCH = D_FF // FC        # 8 chunks
MT = D_MODEL // 128     # 8 output d-tiles

_program_cache = {}


def _col_chunks(C):
    """Split C columns into <=512 pieces, all >=256 when possible (float32r
    runs at 1 cycle/row only for free dim >= 256)."""
    if C <= 512:
        return [(0, C)]
    n = (C + 511) // 512
    base = C // n
    rem = C - base * n
    out = []
    off = 0
    for i in range(n):
        sz = base + (1 if i < rem else 0)
        out.append((off, sz))
        off += sz
    return out


def _build_program(C):
    import concourse.bacc as bacc
    import concourse.mybir as mybir
    from concourse.tile import TileContext

    F32R = mybir.dt.float32r
    F32 = mybir.dt.float32
    SILU = mybir.ActivationFunctionType.Silu
    ccs = _col_chunks(C)

    nc = bacc.Bacc()
    xt_d = nc.declare_dram_parameter("xt", [D_MODEL, C], F32R, isOutput=False)
    w1_d = nc.declare_dram_parameter("w1", [D_MODEL, D_FF], F32R, isOutput=False)
    w2_d = nc.declare_dram_parameter("w2", [D_MODEL, D_FF], F32R, isOutput=False)
    w3_d = nc.declare_dram_parameter("w3", [D_FF, D_MODEL], F32R, isOutput=False)
    ot_d = nc.declare_dram_parameter("ot", [D_MODEL, C], F32, isOutput=True)

    xt_r = xt_d.rearrange("(k p) c -> k p c", p=128)
    w1_r = w1_d.rearrange("(k p) f -> k p f", p=128)
    w2_r = w2_d.rearrange("(k p) f -> k p f", p=128)
    w3_r = w3_d.rearrange("(j p) d -> j p d", p=128)
    ot_r = ot_d.rearrange("(m p) c -> m p c", p=128)

    with TileContext(nc) as tc:
        with (
            tc.tile_pool(name="xtp", bufs=1) as xt_pool,
            tc.tile_pool(name="w12", bufs=2) as w12_pool,
            tc.tile_pool(name="w3p", bufs=2) as w3_pool,
            tc.tile_pool(name="htp", bufs=2) as ht_pool,
            tc.tile_pool(name="otp", bufs=1) as ot_pool,
            tc.tile_pool(name="tmp", bufs=4) as tmp_pool,
            tc.tile_pool(name="pg", bufs=2, space="PSUM") as pg_pool,
            tc.tile_pool(name="pv", bufs=2, space="PSUM") as pv_pool,
            tc.tile_pool(name="po", bufs=3, space="PSUM") as po_pool,
        ):
            xt_sb = [None] * KT
            ot_sb = [
                ot_pool.tile([128, C], F32, tag=f"ot{m}", name=f"ot{m}")
                for m in range(MT)
            ]

            def load_chunk(ch):
                """DMA chunk ch weights in consumption order; chunk 0 also
                interleaves the xt slabs so the first matmul starts ASAP."""
                f0 = ch * FC
                w1c, w2c = [], []
                for k in range(KT):
                    t1 = w12_pool.tile(
                        [128, FC], F32R, tag=f"w1k{k}", name=f"w1c{k}"
                    )
                    if ch == 0:
                        xt_t = xt_pool.tile(
                            [128, C], F32R, tag=f"xt{k}", name=f"xt{k}"
                        )
                        nc.sync.dma_start(out=xt_t[:], in_=xt_r[k])
                        xt_sb[k] = xt_t
                    nc.sync.dma_start(out=t1[:], in_=w1_r[k][:, f0 : f0 + FC])
                    w1c.append(t1)
                for k in range(KT):
                    t2 = w12_pool.tile(
                        [128, FC], F32R, tag=f"w2k{k}", name=f"w2c{k}"
                    )
                    nc.sync.dma_start(out=t2[:], in_=w2_r[k][:, f0 : f0 + FC])
                    w2c.append(t2)
                w3c = []
                for jj in range(JT):
                    t3 = w3_pool.tile(
                        [128, D_MODEL], F32R, tag=f"w3j{jj}", name=f"w3c{jj}"
                    )
                    nc.sync.dma_start(out=t3[:], in_=w3_r[ch * JT + jj])
                    w3c.append(t3)
                return w1c, w2c, w3c

            def phase_a(w1c, w2c):
                """GT/VT matmuls + silu*mul epilogue -> HT tiles for a chunk."""
                hts = []
                for jj in range(JT):
                    ht_t = ht_pool.tile([128, C], F32R, tag=f"ht{jj}", name=f"ht{jj}")
                    js = slice(jj * 128, (jj + 1) * 128)
                    for c0, cl in ccs:
                        cs = slice(c0, c0 + cl)
                        pg = pg_pool.tile([128, cl], F32, tag="pg", name="pg")
                        pv = pv_pool.tile([128, cl], F32, tag="pv", name="pv")
                        for k in range(KT):
                            nc.tensor.matmul(
                                out=pg[:],
                                lhsT=w1c[k][:, js],
                                rhs=xt_sb[k][:, cs],
                                start=(k == 0),
                                stop=(k == KT - 1),
                            )
                        for k in range(KT):
                            nc.tensor.matmul(
                                out=pv[:],
                                lhsT=w2c[k][:, js],
                                rhs=xt_sb[k][:, cs],
                                start=(k == 0),
                                stop=(k == KT - 1),
                            )
                        st = tmp_pool.tile([128, cl], F32, tag="silu", name="st")
                        nc.scalar.activation(st[:], pg[:], SILU)
                        nc.vector.tensor_mul(out=ht_t[:, cs], in0=st[:], in1=pv[:])
                    hts.append(ht_t)
                return hts

            def phase_b(ch, w3c, hts):
                """OT partial accumulation for a chunk; final chunk also
                stores each OT slab as soon as it is complete."""
                for m in range(MT):
                    ms = slice(m * 128, (m + 1) * 128)
                    for c0, cl in ccs:
                        cs = slice(c0, c0 + cl)
                        po = po_pool.tile([128, cl], F32, tag="po", name="po")
                        for jj in range(JT):
                            nc.tensor.matmul(
                                out=po[:],
                                lhsT=w3c[jj][:, ms],
                                rhs=hts[jj][:, cs],
                                start=(jj == 0),
                                stop=(jj == JT - 1),
                            )
                        if ch == 0:
                            nc.vector.tensor_copy(out=ot_sb[m][:, cs], in_=po[:])
                        else:
                            nc.vector.tensor_add(
                                out=ot_sb[m][:, cs], in0=ot_sb[m][:, cs], in1=po[:]
                            )
                    if ch == NCH - 1:
                        nc.sync.dma_start(out=ot_r[m], in_=ot_sb[m][:])

            # software pipeline: B(ch) issues after A(ch+1) so phase B never
            # stalls the PE on the ACT/DVE epilogue producing its HT input
            w1c, w2c, w3c = load_chunk(0)
            hts = phase_a(w1c, w2c)
            prev = (0, w3c, hts)
            for ch in range(1, NCH):
                w1c, w2c, w3c = load_chunk(ch)
                hts = phase_a(w1c, w2c)
                phase_b(*prev)
                prev = (ch, w3c, hts)
            phase_b(*prev)

    nc.compile()
    return nc


def _get_program(C):
    if C not in _program_cache:
        _program_cache[C] = _build_program(C)
    return _program_cache[C]


def _run(nc, in_maps, trace=False):
    from concourse.bass_utils import run_bass_kernel_spmd

    last = None
    for attempt in range(3):
        try:
            return run_bass_kernel_spmd(
                nc, in_maps, list(range(N_EXPERTS)), trace=trace
            )
        except Exception as e:  # stale device state from a prior crash
            last = e
    raise last


def kernel(x, expert_indices, expert_weights, w1, w2, w3, _trace=False):
    x = np.ascontiguousarray(np.asarray(x, dtype=np.float32))
    expert_indices = np.asarray(expert_indices)
    expert_weights = np.asarray(expert_weights, dtype=np.float32)
    w1 = np.asarray(w1, dtype=np.float32)
    w2 = np.asarray(w2, dtype=np.float32)
    w3 = np.asarray(w3, dtype=np.float32)

    n_tokens, d_model = x.shape
    top_k = expert_indices.shape[1]
    n_experts = w1.shape[0]
    A = n_tokens * top_k

    flat_e = expert_indices.reshape(-1).astype(np.int64)
    flat_w = expert_weights.reshape(-1)
    tok_idx = np.repeat(np.arange(n_tokens), top_k)
    order = np.argsort(flat_e, kind="stable")
    s_tok = tok_idx[order]
    s_w = flat_w[order]
    counts = np.bincount(flat_e, minlength=n_experts)
    starts = np.concatenate([[0], np.cumsum(counts)[:-1]])

    C = int(counts.max())
    C = max(256, -(-C // 64) * 64)  # round up to multiple of 64

    xt = np.zeros((n_experts, d_model, C), np.float32)
    for e in range(n_experts):
        seg = s_tok[starts[e] : starts[e] + counts[e]]
        xt[e, :, : counts[e]] = x[seg].T

    nc = _get_program(C)
    in_maps = [
        {"xt": xt[e], "w1": w1[e], "w2": w2[e], "w3": w3[e]}
        for e in range(n_experts)
    ]
    res = _run(nc, in_maps, trace=_trace)

    y = np.empty((A, d_model), np.float32)
    for e in range(n_experts):
        ot = res.results[e]["ot"]
        y[starts[e] : starts[e] + counts[e]] = ot[:, : counts[e]].T
    y *= s_w[:, None]
    y_orig = np.empty_like(y)
    y_orig[order] = y
    out = y_orig.reshape(n_tokens, top_k, d_model).sum(axis=1, dtype=np.float32)
    if _trace:
        return out.astype(np.float32, copy=False), res
    return out.astype(np.float32, copy=False)


# revision 7
# speedup vs baseline: 1.0229x; 1.0093x over previous
"""Batched MoE (top-2, 8 experts) on 8 Trainium2 NeuronCores.

Strategy: expert-parallel — core e owns expert e's weights (w1/w2/w3) and
processes the tokens routed to it. Routing (sort by expert / capacity
padding) and the combine (weighting by gate prob + scatter-add over top-k)
are cheap O(tokens) index ops done on host; all matmul FLOPs run on device.

Device dataflow per core (capacity C columns, zero-padded):
    xt  = X_e^T               [1024, C]   (d on partitions)
    GT  = w1^T @ xt           [4096, C]   lhsT = w1 tiles (natural layout)
    VT  = w2^T @ xt           [4096, C]
    HT  = silu(GT) * VT       [4096, C]
    OT  = w3^T @ HT           [1024, C]   lhsT = w3 tiles (natural layout)
All operands enter the PE in their natural DRAM layout — no transposes.
Matmuls run in float32r (TF32-class, 1 cycle/row at free-dim >= 256, 4x
faster than plain fp32). The f dimension is processed in chunks of 512 so
weights stream through SBUF; OT accumulates across chunks in SBUF via DVE.
Phase B (OT accumulation) of chunk ch is issued after phase A of chunk
ch+1 so the PE never waits on the ACT/DVE epilogue that produces HT.
"""

import numpy as np

N_EXPERTS = 8
D_MODEL = 1024
D_FF = 4096
FC = 512                # d_ff chunk streamed through SBUF
KT = D_MODEL // 128     # 8 k-tiles (contraction d)
JT = FC // 128          # 4 f-tiles per chunk
NCH = D_FF // FC        # 8 chunks
MT = D_MODEL // 128     # 8 output d-tiles

_program_cache = {}


def _col_chunks(C):
    """Split C columns into <=512 pieces, all >=256 when possible (float32r
    runs at 1 cycle/row only for free dim >= 256)."""
    if C <= 512:
        return [(0, C)]
    n = (C + 511) // 512
    base = C // n
    rem = C - base * n
    out = []
    off = 0
    for i in range(n):
        sz = base + (1 if i < rem else 0)
        out.append((off, sz))
        off += sz
    return out


def _build_program(C):
    import concourse.bacc as bacc
    import concourse.mybir as mybir
    from concourse.tile import TileContext

    F32R = mybir.dt.float32r
    F32 = mybir.dt.float32
    SILU = mybir.ActivationFunctionType.Silu
    ccs = _col_chunks(C)

    nc = bacc.Bacc()
    xt_d = nc.declare_dram_parameter("xt", [D_MODEL, C], F32R, isOutput=False)
    w1_d = nc.declare_dram_parameter("w1", [D_MODEL, D_FF], F32R, isOutput=False)
    w2_d = nc.declare_dram_parameter("w2", [D_MODEL, D_FF], F32R, isOutput=False)
    w3_d = nc.declare_dram_parameter("w3", [D_FF, D_MODEL], F32R, isOutput=False)
    ot_d = nc.declare_dram_parameter("ot", [D_MODEL, C], F32, isOutput=True)

    xt_r = xt_d.rearrange("(k p) c -> k p c", p=128)
    w1_r = w1_d.rearrange("(k p) f -> k p f", p=128)
    w2_r = w2_d.rearrange("(k p) f -> k p f", p=128)
    w3_r = w3_d.rearrange("(j p) d -> j p d", p=128)
    ot_r = ot_d.rearrange("(m p) c -> m p c", p=128)

    with TileContext(nc) as tc:
        with (
            tc.tile_pool(name="xtp", bufs=1) as xt_pool,
            tc.tile_pool(name="w12", bufs=2) as w12_pool,
            tc.tile_pool(name="w3p", bufs=2) as w3_pool,
            tc.tile_pool(name="htp", bufs=2) as ht_pool,
            tc.tile_pool(name="otp", bufs=1) as ot_pool,
            tc.tile_pool(name="tmp", bufs=4) as tmp_pool,
            tc.tile_pool(name="pg", bufs=2, space="PSUM") as pg_pool,
            tc.tile_pool(name="pv", bufs=2, space="PSUM") as pv_pool,
            tc.tile_pool(name="po", bufs=3, space="PSUM") as po_pool,
        ):
            xt_sb = [None] * KT
            ot_sb = [
                ot_pool.tile([128, C], F32, tag=f"ot{m}", name=f"ot{m}")
                for m in range(MT)
            ]

            def load_chunk(ch):
                """DMA chunk ch weights in consumption order; chunk 0 also
                interleaves the xt slabs so the first matmul starts ASAP."""
                f0 = ch * FC
                w1c, w2c = [], []
                for k in range(KT):
                    t1 = w12_pool.tile(
                        [128, FC], F32R, tag=f"w1k{k}", name=f"w1c{k}"
                    )
                    if ch == 0:
                        # xt goes on the ACT HWDGE ring so it streams in
                        # parallel with the weight ring at startup
                        xt_t = xt_pool.tile(
                            [128, C], F32R, tag=f"xt{k}", name=f"xt{k}"
                        )
                        nc.scalar.dma_start(out=xt_t[:], in_=xt_r[k])
                        xt_sb[k] = xt_t
                    nc.sync.dma_start(out=t1[:], in_=w1_r[k][:, f0 : f0 + FC])
                    w1c.append(t1)
                for k in range(KT):
                    t2 = w12_pool.tile(
                        [128, FC], F32R, tag=f"w2k{k}", name=f"w2c{k}"
                    )
                    nc.sync.dma_start(out=t2[:], in_=w2_r[k][:, f0 : f0 + FC])
                    w2c.append(t2)
                w3c = []
                for jj in range(JT):
                    t3 = w3_pool.tile(
                        [128, D_MODEL], F32R, tag=f"w3j{jj}", name=f"w3c{jj}"
                    )
                    nc.sync.dma_start(out=t3[:], in_=w3_r[ch * JT + jj])
                    w3c.append(t3)
                return w1c, w2c, w3c

            def phase_a(w1c, w2c):
                """GT/VT matmuls + silu*mul epilogue -> HT tiles for a chunk."""
                hts = []
                for jj in range(JT):
                    ht_t = ht_pool.tile([128, C], F32R, tag=f"ht{jj}", name=f"ht{jj}")
                    js = slice(jj * 128, (jj + 1) * 128)
                    for c0, cl in ccs:
                        cs = slice(c0, c0 + cl)
                        pg = pg_pool.tile([128, cl], F32, tag="pg", name="pg")
                        pv = pv_pool.tile([128, cl], F32, tag="pv", name="pv")
                        for k in range(KT):
                            nc.tensor.matmul(
                                out=pg[:],
                                lhsT=w1c[k][:, js],
                                rhs=xt_sb[k][:, cs],
                                start=(k == 0),
                                stop=(k == KT - 1),
                            )
                        for k in range(KT):
                            nc.tensor.matmul(
                                out=pv[:],
                                lhsT=w2c[k][:, js],
                                rhs=xt_sb[k][:, cs],
                                start=(k == 0),
                                stop=(k == KT - 1),
                            )
                        st = tmp_pool.tile([128, cl], F32, tag="silu", name="st")
                        nc.scalar.activation(st[:], pg[:], SILU)
                        nc.vector.tensor_mul(out=ht_t[:, cs], in0=st[:], in1=pv[:])
                    hts.append(ht_t)
                return hts

            def phase_b_m(ch, w3c, hts, m):
                """OT partial accumulation for one output d-tile of a chunk."""
                ms = slice(m * 128, (m + 1) * 128)
                for c0, cl in ccs:
                    cs = slice(c0, c0 + cl)
                    po = po_pool.tile([128, cl], F32, tag="po", name="po")
                    for jj in range(JT):
                        nc.tensor.matmul(
                            out=po[:],
                            lhsT=w3c[jj][:, ms],
                            rhs=hts[jj][:, cs],
                            start=(jj == 0),
                            stop=(jj == JT - 1),
                        )
                    if ch == 0:
                        nc.vector.tensor_copy(out=ot_sb[m][:, cs], in_=po[:])
                    else:
                        nc.vector.tensor_add(
                            out=ot_sb[m][:, cs], in0=ot_sb[m][:, cs], in1=po[:]
                        )

            def phase_b(ch, w3c, hts):
                for m in range(MT):
                    phase_b_m(ch, w3c, hts, m)

            # software pipeline: B(ch) issues after A(ch+1) so phase B never
            # stalls the PE on the ACT/DVE epilogue producing its HT input.
            # The last two B passes interleave m-wise so each OT slab's store
            # DMA overlaps the remaining matmuls instead of draining at the
            # very end.
            w1c, w2c, w3c = load_chunk(0)
            hts = phase_a(w1c, w2c)
            prev = (0, w3c, hts)
            for ch in range(1, NCH):
                w1c, w2c, w3c = load_chunk(ch)
                hts = phase_a(w1c, w2c)
                if ch < NCH - 1:
                    phase_b(*prev)
                    prev = (ch, w3c, hts)
            for m in range(MT):
                phase_b_m(prev[0], prev[1], prev[2], m)
                phase_b_m(NCH - 1, w3c, hts, m)
                nc.sync.dma_start(out=ot_r[m], in_=ot_sb[m][:])

    nc.compile()
    return nc


def _get_program(C):
    if C not in _program_cache:
        _program_cache[C] = _build_program(C)
    return _program_cache[C]


def _run(nc, in_maps, trace=False):
    from concourse.bass_utils import run_bass_kernel_spmd

    last = None
    for attempt in range(3):
        try:
            return run_bass_kernel_spmd(
                nc, in_maps, list(range(N_EXPERTS)), trace=trace
            )
        except Exception as e:  # stale device state from a prior crash
            last = e
    raise last


def kernel(x, expert_indices, expert_weights, w1, w2, w3, _trace=False):
    x = np.ascontiguousarray(np.asarray(x, dtype=np.float32))
    expert_indices = np.asarray(expert_indices)
    expert_weights = np.asarray(expert_weights, dtype=np.float32)
    w1 = np.asarray(w1, dtype=np.float32)
    w2 = np.asarray(w2, dtype=np.float32)
    w3 = np.asarray(w3, dtype=np.float32)

    n_tokens, d_model = x.shape
    top_k = expert_indices.shape[1]
    n_experts = w1.shape[0]
    A = n_tokens * top_k

    flat_e = expert_indices.reshape(-1).astype(np.int64)
    flat_w = expert_weights.reshape(-1)
    tok_idx = np.repeat(np.arange(n_tokens), top_k)
    order = np.argsort(flat_e, kind="stable")
    s_tok = tok_idx[order]
    s_w = flat_w[order]
    counts = np.bincount(flat_e, minlength=n_experts)
    starts = np.concatenate([[0], np.cumsum(counts)[:-1]])

    C = int(counts.max())
    C = max(256, -(-C // 64) * 64)  # round up to multiple of 64

    xt = np.zeros((n_experts, d_model, C), np.float32)
    for e in range(n_experts):
        seg = s_tok[starts[e] : starts[e] + counts[e]]
        xt[e, :, : counts[e]] = x[seg].T

    nc = _get_program(C)
    in_maps = [
        {"xt": xt[e], "w1": w1[e], "w2": w2[e], "w3": w3[e]}
        for e in range(n_experts)
    ]
    res = _run(nc, in_maps, trace=_trace)

    y = np.empty((A, d_model), np.float32)
    for e in range(n_experts):
        ot = res.results[e]["ot"]
        y[starts[e] : starts[e] + counts[e]] = ot[:, : counts[e]].T
    y *= s_w[:, None]
    y_orig = np.empty_like(y)
    y_orig[order] = y
    out = y_orig.reshape(n_tokens, top_k, d_model).sum(axis=1, dtype=np.float32)
    if _trace:
        return out.astype(np.float32, copy=False), res
    return out.astype(np.float32, copy=False)
